# revision 11
# baseline (speedup 1.0000x reference)
"""Trainium2 Bass kernel for nn_PartialAttention (LN -> Q/K proj -> scaled QK^T -> exp(s - rowmax)).

Sharding: 8 cores = 2 batches x 4 query-blocks of 1024 queries.
Each core receives the full batch sequence in transposed layout xT = X_b^T
[E=1024, S=4096], column-rolled so that its own query block occupies
columns 0..1023 (keeps the device program identical across cores).
The core computes LayerNorm statistics + K^T for the whole batch and
Q^T for its block via the decomposition

    K^T = r (.) (Wg_k^T xT) - sk (x) (r*mu) + ck (x) 1,   Wg_k = diag(gamma) Wk

then scores = Q^T.T @ K^T and out = exp(scores - rowmax).
The host un-rolls the key axis of each block and concatenates.
"""

import os
from contextlib import ExitStack

import numpy as np

import concourse.bass as bass
import concourse.bacc as bacc
import concourse.mybir as mybir
import concourse.tile as tile
from concourse.bass import ts
from concourse.bass_utils import run_bass_kernel_spmd

F32 = mybir.dt.float32
F32R = mybir.dt.float32r
FT = mybir.ActivationFunctionType
AX = mybir.AxisListType

E, S, B, D = 1024, 4096, 2, 64
P = 128
NE = E // P            # 8 e-chunks of 128
TS = 512               # token chunk (= one fp32 PSUM bank)
NTS = S // TS          # 8
QB = 1024              # queries per core
NQC = QB // TS         # 2 ts-chunks belong to the query block
NQT = QB // P          # 8 query tiles of 128
EPS = 1e-5
SCALE = 1.0 / 8.0      # 1/sqrt(D)

# Matmul dtype knob: F32R runs 4x faster on the PE (1 cyc/row vs 4) at
# reduced multiply precision; F32 is the full-precision fallback.
MM_DT = F32R
# How many of the 8 e-chunks each engine squares: (scalar, vector, gpsimd)
SQ_SPLIT = (3, 3, 2)


def _mm(ap):
    return ap.bitcast(MM_DT) if MM_DT is not F32 else ap


def _body(tc, xT, wq, wk, gam, bet, bqv, bkv, cst, cstn, out):
    nc = tc.nc
    with ExitStack() as ctx:
        consts = ctx.enter_context(tc.tile_pool(name="consts", bufs=1))
        big = ctx.enter_context(tc.tile_pool(name="big", bufs=1))
        stats = ctx.enter_context(tc.tile_pool(name="stats", bufs=1))

        # ---------- parameter prep ----------
        wkt = consts.tile([P, NE, D], MM_DT)
        nc.gpsimd.dma_start(out=wkt, in_=_mm(wk.rearrange("(c p) d -> p c d", p=P)))
        wqt = consts.tile([P, NE, D], MM_DT)
        nc.gpsimd.dma_start(out=wqt, in_=_mm(wq.rearrange("(c p) d -> p c d", p=P)))
        gmt = consts.tile([P, NE], F32)
        nc.gpsimd.dma_start(out=gmt, in_=gam.rearrange("(c p) -> p c", p=P))
        btt = consts.tile([P, NE], MM_DT)
        nc.gpsimd.dma_start(out=btt, in_=_mm(bet.rearrange("(c p) -> p c", p=P)))
        bk_row = consts.tile([1, D], F32)
        nc.gpsimd.dma_start(out=bk_row, in_=bkv.unsqueeze(0))
        bq_row = consts.tile([1, D], F32)
        nc.gpsimd.dma_start(out=bq_row, in_=bqv.unsqueeze(0))

        wgk = consts.tile([P, NE, D], MM_DT)
        wgq = consts.tile([P, NE, D], MM_DT)
        for c in range(NE):
            nc.vector.tensor_scalar_mul(wgk[:, c, :], wkt[:, c, :], gmt[:, c : c + 1])
            nc.vector.tensor_scalar(
                wgq[:, c, :],
                wqt[:, c, :],
                gmt[:, c : c + 1],
                SCALE,
                op0=mybir.AluOpType.mult,
                op1=mybir.AluOpType.mult,
            )

        # Constant operands for FP32R matmuls are DMA'd from the host-supplied
        # cst tensor (memset cannot write float32r).
        # cst[:, 0:15] = staircase (col NTS-1 ones), cst[:, 15] = ones.
        stair_ones = consts.tile([P, 2 * NTS], MM_DT)
        nc.gpsimd.dma_start(out=stair_ones, in_=_mm(cst))
        stair = stair_ones[:, 0 : 2 * NTS - 1]
        ones_col = stair_ones[:, 2 * NTS - 1 : 2 * NTS]
        negones = consts.tile([1, TS], MM_DT)
        nc.gpsimd.dma_start(out=negones, in_=_mm(cstn))

        # sk/sq/ck/cq rows [1, D] via PE column sums
        sk_row = consts.tile([1, D], MM_DT)
        sq_row = consts.tile([1, D], MM_DT)
        ck_row = consts.tile([1, D], MM_DT)
        cq_row = consts.tile([1, D], MM_DT)
        with tc.tile_pool(name="ppsum", bufs=1, space="PSUM") as pp:
            ps_par = pp.tile([1, 4 * D], F32)
            for g in range(4):
                rhs_t = (wgk, wgq, wkt, wqt)[g]
                for c in range(NE):
                    lhs = ones_col if g < 2 else btt[:, c : c + 1]
                    nc.tensor.matmul(ps_par[:, g * D : (g + 1) * D], lhsT=lhs, rhs=rhs_t[:, c, :], start=(c == 0), stop=(c == NE - 1), skip_group_check=True)
            nc.scalar.copy(sk_row, ps_par[:, 0 * D : 1 * D])
            nc.scalar.copy(sq_row, ps_par[:, 1 * D : 2 * D])
            nc.vector.tensor_add(ck_row, ps_par[:, 2 * D : 3 * D], bk_row)
            tmpc = stats.tile([1, D], F32)
            nc.vector.tensor_add(tmpc, ps_par[:, 3 * D : 4 * D], bq_row)
            nc.vector.tensor_scalar_mul(cq_row, tmpc, SCALE)

        # ---------- phase 1: stream x, projections + raw stats ----------
        pkraw = big.tile([D, S], F32)
        pqraw = big.tile([D, QB], F32)
        xT3 = xT.rearrange("(c p) t -> p c t", p=P)
        a0, a1, _ = SQ_SPLIT
        with (
            tc.tile_pool(name="xpool", bufs=2) as xpool,
            tc.tile_pool(name="sqpool", bufs=2) as sqpool,
            tc.tile_pool(name="kp", bufs=2, space="PSUM") as kp,
            tc.tile_pool(name="qp", bufs=2, space="PSUM") as qp,
            tc.tile_pool(name="sp", bufs=1, space="PSUM") as sp,
        ):
            ps_s1 = sp.tile([NTS, TS], F32)
            ps_s2 = sp.tile([NTS, TS], F32)
            for j in range(NTS):
                xt = xpool.tile([P, NE, TS], MM_DT)
                nc.sync.dma_start(out=xt, in_=_mm(xT3[:, :, ts(j, TS)]))
                xq2 = sqpool.tile([P, NE, TS], MM_DT)
                nc.scalar.square(xq2[:, 0:a0, :], xt[:, 0:a0, :])
                nc.vector.tensor_mul(xq2[:, a0 : a0 + a1, :], xt[:, a0 : a0 + a1, :], xt[:, a0 : a0 + a1, :])
                nc.gpsimd.tensor_mul(xq2[:, a0 + a1 :, :], xt[:, a0 + a1 :, :], xt[:, a0 + a1 :, :])

                pk = kp.tile([D, TS], F32)
                for c in range(NE):
                    nc.tensor.matmul(pk, lhsT=wgk[:, c, :], rhs=xt[:, c, :], start=(c == 0), stop=(c == NE - 1))
                nc.scalar.copy(pkraw[:, ts(j, TS)], pk)
                if j < NQC:
                    pq = qp.tile([D, TS], F32)
                    for c in range(NE):
                        nc.tensor.matmul(pq, lhsT=wgq[:, c, :], rhs=xt[:, c, :], start=(c == 0), stop=(c == NE - 1))
                    nc.scalar.copy(pqraw[:, ts(j, TS)], pq)

                lhs_st = stair[:, NTS - 1 - j : 2 * NTS - 1 - j]
                for c in range(NE):
                    nc.tensor.matmul(ps_s1, lhsT=lhs_st, rhs=xt[:, c, :], start=(j == 0 and c == 0), stop=(j == NTS - 1 and c == NE - 1), skip_group_check=True)
                for c in range(NE):
                    nc.tensor.matmul(ps_s2, lhsT=lhs_st, rhs=xq2[:, c, :], start=(j == 0 and c == 0), stop=(j == NTS - 1 and c == NE - 1), skip_group_check=True)

            # ---------- phase 1.5: stats ----------
            mu8 = stats.tile([NTS, TS], F32)
            nc.vector.tensor_scalar_mul(mu8, ps_s1, 1.0 / E)
            e28 = stats.tile([NTS, TS], F32)
            nc.vector.tensor_scalar_mul(e28, ps_s2, 1.0 / E)
            msq8 = stats.tile([NTS, TS], F32)
            nc.vector.tensor_mul(msq8, mu8, mu8)
            var8 = stats.tile([NTS, TS], F32)
            nc.vector.tensor_sub(var8, e28, msq8)
            eps8 = stats.tile([NTS, 1], F32)
            nc.vector.memset(eps8, EPS)
            sd8 = stats.tile([NTS, TS], F32)
            nc.scalar.activation(sd8, var8, FT.Sqrt, bias=eps8[:, 0:1])
            r8 = stats.tile([NTS, TS], F32)
            nc.vector.reciprocal(r8, sd8)
            rmu8 = stats.tile([NTS, TS], F32)
            nc.vector.tensor_mul(rmu8, r8, mu8)

        rmu_row = stats.tile([1, S], MM_DT)
        nc.sync.dma_start(out=rmu_row, in_=_mm(rmu8))
        # SBUF APs cannot broadcast across partitions; bounce r through DRAM
        # and broadcast-read it back (partition step 0 is legal on DRAM APs).
        r_dram = nc.dram_tensor("r_scratch", [S], F32).ap()
        nc.sync.dma_start(out=r_dram, in_=r8)
        rb = big.tile([D, S], F32)
        r_bcast = bass.AP(tensor=r_dram.tensor, offset=r_dram.offset, ap=[[0, D]] + list(r_dram.ap))
        nc.sync.dma_start(out=rb, in_=r_bcast)

        # ---------- KT / QT epilogue ----------
        kT = big.tile([D, S], MM_DT)
        qT = big.tile([D, QB], MM_DT)
        with (
            tc.tile_pool(name="ep", bufs=2, space="PSUM") as ep,
            tc.tile_pool(name="ktmp", bufs=2) as ktmp_pool,
        ):
            for j in range(NTS):
                ob = ep.tile([D, TS], F32)
                nc.tensor.matmul(ob, lhsT=sk_row, rhs=rmu_row[:, ts(j, TS)], start=True, stop=False)
                nc.tensor.matmul(ob, lhsT=ck_row, rhs=negones, start=False, stop=True)
                tmp = ktmp_pool.tile([D, TS], F32)
                nc.vector.tensor_mul(tmp, rb[:, ts(j, TS)], pkraw[:, ts(j, TS)])
                nc.vector.tensor_sub(kT[:, ts(j, TS)], tmp, ob)
                if j < NQC:
                    obq = ep.tile([D, TS], F32)
                    nc.tensor.matmul(obq, lhsT=sq_row, rhs=rmu_row[:, ts(j, TS)], start=True, stop=False)
                    nc.tensor.matmul(obq, lhsT=cq_row, rhs=negones, start=False, stop=True)
                    tmpq = ktmp_pool.tile([D, TS], F32)
                    nc.vector.tensor_mul(tmpq, rb[:, ts(j, TS)], pqraw[:, ts(j, TS)])
                    nc.vector.tensor_sub(qT[:, ts(j, TS)], tmpq, obq)

        # ---------- phase 2: scores + rowmax + exp ----------
        with (
            tc.tile_pool(name="scorep", bufs=8, space="PSUM") as scorep,
            tc.tile_pool(name="outp", bufs=2) as outp,
            tc.tile_pool(name="mxp", bufs=2) as mxp,
        ):
            for m in range(NQT):
                o_t = outp.tile([P, S], F32)
                mx8 = mxp.tile([P, NTS], F32)
                banks = []
                for j in range(NTS):
                    ps = scorep.tile([P, TS], F32, name=f"s{m}_{j}", tag="s")
                    nc.tensor.matmul(ps, lhsT=qT[:, ts(m, P)], rhs=kT[:, ts(j, TS)], start=True, stop=True)
                    nc.vector.reduce_max(mx8[:, j : j + 1], ps, axis=AX.X)
                    banks.append(ps)
                nmax = mxp.tile([P, 1], F32)
                nc.vector.reduce_max(nmax, mx8, axis=AX.X, negate=True)
                for j in range(NTS):
                    nc.scalar.activation(o_t[:, ts(j, TS)], banks[j], FT.Exp, bias=nmax[:, 0:1])
                nc.sync.dma_start(out=out[ts(m, P), :], in_=o_t)


def _build_nc():
    nc = bacc.Bacc("TRN2", target_bir_lowering=False, debug=False)
    xT = nc.dram_tensor("xT", [E, S], F32, kind="ExternalInput").ap()
    wq = nc.dram_tensor("Wq", [E, D], F32, kind="ExternalInput").ap()
    wk = nc.dram_tensor("Wk", [E, D], F32, kind="ExternalInput").ap()
    gam = nc.dram_tensor("gamma", [E], F32, kind="ExternalInput").ap()
    bet = nc.dram_tensor("beta", [E], F32, kind="ExternalInput").ap()
    bqv = nc.dram_tensor("bq", [D], F32, kind="ExternalInput").ap()
    bkv = nc.dram_tensor("bk", [D], F32, kind="ExternalInput").ap()
    cst = nc.dram_tensor("cst", [P, 2 * NTS], F32, kind="ExternalInput").ap()
    cstn = nc.dram_tensor("cstn", [1, TS], F32, kind="ExternalInput").ap()
    out = nc.dram_tensor("out", [QB, S], F32, kind="ExternalOutput").ap()
    with tile.TileContext(nc) as tc:
        _body(tc, xT, wq, wk, gam, bet, bqv, bkv, cst, cstn, out)
    nc.compile()
    return nc


_nc_cache = None
_last_results = None


def kernel(src_emb, gamma, beta, Wq, bq, Wk, bk):
    global _nc_cache, _last_results
    src_emb = np.asarray(src_emb, np.float32)
    gamma = np.asarray(gamma, np.float32)
    beta = np.asarray(beta, np.float32)
    Wq = np.asarray(Wq, np.float32)
    bq = np.asarray(bq, np.float32)
    Wk = np.asarray(Wk, np.float32)
    bk = np.asarray(bk, np.float32)

    if _nc_cache is None:
        _nc_cache = _build_nc()
    nc = _nc_cache

    cst_np = np.zeros((P, 2 * NTS), np.float32)
    cst_np[:, NTS - 1] = 1.0
    cst_np[:, 2 * NTS - 1] = 1.0
    cstn_np = np.full((1, TS), -1.0, np.float32)
    xT_all = np.ascontiguousarray(np.transpose(src_emb, (1, 2, 0)))  # [B, E, S]
    in_maps = []
    for c in range(8):
        b, qb = c // 4, c % 4
        s = qb * QB
        if s:
            xr = np.ascontiguousarray(np.concatenate([xT_all[b][:, s:], xT_all[b][:, :s]], axis=1))
        else:
            xr = xT_all[b]
        in_maps.append({"xT": xr, "Wq": Wq, "Wk": Wk, "gamma": gamma, "beta": beta, "bq": bq, "bk": bk, "cst": cst_np, "cstn": cstn_np})

    res = run_bass_kernel_spmd(nc, in_maps, core_ids=list(range(8)))
    _last_results = res

    blocks = []
    for c in range(8):
        blk = res.results[c]["out"]
        s = (c % 4) * QB
        if s:
            blk = np.roll(blk, s, axis=1)
        blocks.append(blk)
    return np.stack(
        [np.concatenate(blocks[0:4], axis=0), np.concatenate(blocks[4:8], axis=0)], axis=0
    )
